# revision 8
# baseline (speedup 1.0000x reference)
"""Trainium2 Bass kernel for the BLIPv2 agent-attention block.

Full (unsharded) inputs in, full outputs out. Data-parallel over the batch
axis across 8 NeuronCores; parameters replicated.

Per-core program (64 batch items, 65 tokens, dim 768):
  Phase A: LN1 -> qkv (f32r matmuls) -> per-item agent attention (bf16
           matmuls) + depthwise 3x3 conv (DVE) -> proj -> xa = x + att
           (bounced to DRAM).
  Phase B: LN2 -> fc1 (f32r) -> exact gelu (ACT) -> fc2 (bf16) -> out =
           xa + mlp.
Layout: token-major for LayerNorm / residuals, feature-major for matmuls,
PE transposes to convert. Buffers are tiled per item-aligned column group
(4 items = 260 columns) so LN / qkv / attention / proj pipeline.
"""

import sys
from contextlib import ExitStack

import numpy as np

sys.path.insert(0, "/opt/trn_rl_repo")

DIM = 768
KC = 6            # DIM / 128
NTOK = 65
AG = 49
MLP = 3072
MC = 24           # MLP / 128
NCORES = 8
NITEMS = 64       # batch items per core
NB = 8            # items per sub-batch
NSB = NITEMS // NB
INT = 4           # items per column group (N-tile)
NTC = INT * NTOK  # N-tile columns (260): >=256 keeps f32r at full rate
NNT = NB // INT   # N-tiles per sub-batch (2)
TSB = NB * NTOK   # tokens per sub-batch (520)
TOKS = NITEMS * NTOK  # tokens per core (4160)
EPS = 1e-5
SCALE4 = (DIM ** -0.5) / 4.0  # folds the 2x2-avg-pool 1/4 into the softmax scale

_PROG = None


def _ntiles(total, step):
    out = []
    o = 0
    while o < total:
        out.append((o, min(step, total - o)))
        o += step
    return out


def _pieces(lo, hi):
    """Split token range [lo, hi) at N-tile boundaries -> (ntile, start, len)."""
    out = []
    while lo < hi:
        nt = lo // NTC
        end = min(hi, (nt + 1) * NTC)
        out.append((nt, lo - nt * NTC, end - lo))
        lo = end
    return out


def _build_program():
    import concourse.mybir as mybir
    from concourse import bacc
    from concourse.tile import TileContext
    from concourse.masks import make_identity

    f32 = mybir.dt.float32
    f32r = mybir.dt.float32r
    bf16 = mybir.dt.bfloat16
    AF = mybir.ActivationFunctionType
    OP = mybir.AluOpType
    AX = mybir.AxisListType

    nc = bacc.Bacc("TRN2", target_bir_lowering=False, debug=False,
                   num_devices=NCORES)

    x_d = nc.dram_tensor("x", [TOKS, DIM], f32, kind="ExternalInput")
    wqkv_d = nc.dram_tensor("wqkvT", [DIM, 3 * DIM], f32, kind="ExternalInput")
    wproj_d = nc.dram_tensor("wprojT", [DIM, DIM], f32, kind="ExternalInput")
    fc1_d = nc.dram_tensor("fc1T", [DIM, MLP], f32, kind="ExternalInput")
    fc2_d = nc.dram_tensor("fc2T", [MLP, DIM], f32, kind="ExternalInput")
    ln1g_d = nc.dram_tensor("ln1g", [128, KC], f32, kind="ExternalInput")
    ln1b_d = nc.dram_tensor("ln1b", [128, KC], f32, kind="ExternalInput")
    ln2g_d = nc.dram_tensor("ln2g", [128, KC], f32, kind="ExternalInput")
    ln2b_d = nc.dram_tensor("ln2b", [128, KC], f32, kind="ExternalInput")
    bproj_d = nc.dram_tensor("bproj", [128, KC], f32, kind="ExternalInput")
    fc1b_d = nc.dram_tensor("fc1b", [128, MC], f32, kind="ExternalInput")
    fc2b_d = nc.dram_tensor("fc2b", [128, KC], f32, kind="ExternalInput")
    dwcw_d = nc.dram_tensor("dwcw", [128, KC * 9], f32, kind="ExternalInput")
    dwcb_d = nc.dram_tensor("dwcb", [128, KC], f32, kind="ExternalInput")
    pbc_d = nc.dram_tensor("pbc", [AG, NTOK], f32, kind="ExternalInput")
    abc_d = nc.dram_tensor("abc", [NTOK, AG], f32, kind="ExternalInput")

    xa_d = nc.dram_tensor("xa", [TOKS, DIM], f32)
    xout_d = nc.dram_tensor("xout", [TOKS, DIM], f32, kind="ExternalOutput")
    rep_d = nc.dram_tensor("rep", [NITEMS, AG, NTOK], f32, kind="ExternalOutput")

    xap = x_d.ap()
    xaap = xa_d.ap()
    xoap = xout_d.ap()
    repap = rep_d.ap()

    with TileContext(nc) as tc, ExitStack() as _stk:
        cns = _stk.enter_context(tc.tile_pool(name="consts", bufs=1))
        identf = cns.tile([128, 128], f32)
        make_identity(nc, identf[:, :])
        identr = cns.tile([128, 128], f32r)
        nc.vector.tensor_copy(identr[:, :], identf[:, :])
        identb = cns.tile([128, 128], bf16)
        make_identity(nc, identb[:, :])
        eps_t = cns.tile([128, 1], f32)
        nc.vector.memset(eps_t[:, :], EPS)

        def _load_const(dram, shape):
            t = cns.tile(shape, f32, name=dram.name + "_t")
            nc.sync.dma_start(out=t[...], in_=dram.ap())
            return t

        ln1g_t = _load_const(ln1g_d, [128, KC])
        ln1b_t = _load_const(ln1b_d, [128, KC])
        ln2g_t = _load_const(ln2g_d, [128, KC])
        ln2b_t = _load_const(ln2b_d, [128, KC])
        bproj_t = _load_const(bproj_d, [128, KC])
        fc1b_t = _load_const(fc1b_d, [128, MC])
        fc2b_t = _load_const(fc2b_d, [128, KC])
        dwcw_t = _load_const(dwcw_d, [128, KC * 9])
        dwcb_t = _load_const(dwcb_d, [128, KC])
        pbc_t = _load_const(pbc_d, [AG, NTOK])
        abc_t = _load_const(abc_d, [NTOK, AG])

        def layernorm_to_fm(wkp, psp, src_slice, tp, g_t, b_t, write_fn,
                            ident):
            """LN over one token tile (tp tokens) + transpose to fm chunks.
            write_fn(k, piece_off_in_tile, psum_ap) writes the fm columns."""
            x_tm = wkp.tile([128, DIM], f32, tag="x_tm", bufs=3, name="x_tm")
            nc.sync.dma_start(out=x_tm[:tp, :], in_=src_slice)
            stats = wkp.tile([128, 3, 6], f32, tag="stats", bufs=2,
                             name="stats")
            xg = x_tm[:tp, :].rearrange("p (g d) -> p g d", g=3)
            for g in range(3):
                nc.vector.bn_stats(out=stats[:tp, g, :], in_=xg[:, g, :])
            mv = wkp.tile([128, 2], f32, tag="mv", bufs=2, name="mv")
            nc.vector.bn_aggr(out=mv[:tp, :], in_=stats[:tp, :, :])
            rstd = wkp.tile([128, 1], f32, tag="rstd", bufs=2, name="rstd")
            nc.scalar.activation(rstd[:tp, :], mv[:tp, 1:2], AF.Sqrt,
                                 bias=eps_t[:tp, :])
            nc.vector.reciprocal(rstd[:tp, :], rstd[:tp, :])
            xh_tm = wkp.tile([128, DIM], f32r, tag="xh_tm", bufs=2,
                             name="xh_tm")
            nc.vector.tensor_scalar(out=xh_tm[:tp, :], in0=x_tm[:tp, :],
                                    scalar1=mv[:tp, 0:1], scalar2=rstd[:tp, :],
                                    op0=OP.subtract, op1=OP.mult)
            for k in range(KC):
                tps = psp.tile([128, 128], f32r, tag="tp", bufs=2, name="tps")
                nc.tensor.transpose(tps[:, :tp],
                                    xh_tm[:tp, k * 128:(k + 1) * 128],
                                    ident[:tp, :tp])
                write_fn(k, tps)

        # ---------------- Phase A ----------------
        pwa = tc.tile_pool(name="pwa", bufs=1)
        pbig = tc.tile_pool(name="pbig", bufs=1)
        pwk = tc.tile_pool(name="pwk", bufs=1)
        pps = tc.tile_pool(name="pps", bufs=1, space="PSUM")
        with pwa as wA, pbig as big, pwk as wk, pps as ps:
            wqkv_t = wA.tile([128, KC, 3 * DIM], f32r)
            wproj_t = wA.tile([128, KC, DIM], f32r)
            wq_r = wqkv_d.ap().rearrange("(k p) m -> k p m", p=128)
            wp_r = wproj_d.ap().rearrange("(k p) m -> k p m", p=128)
            for k in range(KC):
                for pc in range(3):
                    st = wk.tile([128, DIM], f32, tag="stage", bufs=2,
                                 name="st")
                    nc.sync.dma_start(out=st[:, :],
                                      in_=wq_r[k][:, pc * DIM:(pc + 1) * DIM])
                    nc.vector.tensor_copy(
                        wqkv_t[:, k, pc * DIM:(pc + 1) * DIM], st[:, :])
                st2 = wk.tile([128, DIM], f32, tag="stage", bufs=2, name="st2")
                nc.sync.dma_start(out=st2[:, :], in_=wp_r[k])
                nc.vector.tensor_copy(wproj_t[:, k, :], st2[:, :])

            for sb in range(NSB):
                t0sb = sb * TSB

                # -- A-i: LN1 + transpose into per-N-tile fm buffers --
                xh_nt = [big.tile([128, KC, NTC], f32r, tag="xy", bufs=3,
                                  name=f"xh{sb}_{n}") for n in range(NNT)]
                for ntile in range(NNT):
                    for (c0, tp) in _ntiles(NTC, 128):
                        def wr(k, tps, ntile=ntile, c0=c0, tp=tp):
                            nc.vector.tensor_scalar(
                                out=xh_nt[ntile][:, k, c0:c0 + tp],
                                in0=tps[:, :tp],
                                scalar1=ln1g_t[:, k:k + 1],
                                scalar2=ln1b_t[:, k:k + 1],
                                op0=OP.mult, op1=OP.add)

                        t0 = t0sb + ntile * NTC + c0
                        layernorm_to_fm(wk, ps, xap[t0: t0 + tp, :], tp,
                                        ln1g_t, ln1b_t, wr, identr)

                # -- A-ii: qkv matmuls (f32r) -> per-N-tile q/k/v bf16 --
                q_nt = [big.tile([128, KC, NTC], bf16, tag="q", bufs=2,
                                 name=f"q{sb}_{n}") for n in range(NNT)]
                k_nt = [big.tile([128, KC, NTC], bf16, tag="k", bufs=2,
                                 name=f"k{sb}_{n}") for n in range(NNT)]
                v_nt = [big.tile([128, KC, NTC], bf16, tag="v", bufs=2,
                                 name=f"v{sb}_{n}") for n in range(NNT)]
                for ntile in range(NNT):
                    for m in range(3 * KC):
                        mm = ps.tile([128, NTC], f32, tag="mm", bufs=2,
                                     name="mmq")
                        for k in range(KC):
                            nc.tensor.matmul(
                                mm[:, :],
                                wqkv_t[:, k, m * 128:(m + 1) * 128],
                                xh_nt[ntile][:, k, :],
                                start=(k == 0), stop=(k == KC - 1))
                        dst = (q_nt, k_nt, v_nt)[m // KC][ntile]
                        nc.any.tensor_copy(dst[:, m % KC, :], mm[:, :])

                # -- A-iii: depthwise 3x3 conv per N-tile --
                dw_nt = []
                for ntile in range(NNT):
                    dw = big.tile([128, KC, INT, NTOK], bf16, tag="dw",
                                  bufs=2, name=f"dw{sb}_{ntile}")
                    dw_nt.append(dw)
                    nc.vector.memset(dw[...], 0.0)
                    for k in range(KC):
                        vsp = v_nt[ntile][:, k, :].rearrange(
                            "p (i n) -> p i n", i=INT)
                        vgrid = vsp[:, :, 1:NTOK].rearrange(
                            "p i (y x) -> p i y x", y=8)
                        dgrid = dw[:, k, :, 1:NTOK].rearrange(
                            "p i (y x) -> p i y x", y=8)
                        for dy in (-1, 0, 1):
                            for dx in (-1, 0, 1):
                                ny, nx = 8 - abs(dy), 8 - abs(dx)
                                oy, ox = max(0, -dy), max(0, -dx)
                                iy, ix = max(0, dy), max(0, dx)
                                tap = (dy + 1) * 3 + (dx + 1)
                                tmp = wk.tile([128, INT, 8, 8], bf16,
                                              tag="dwtmp", bufs=2, name="dwt")
                                nc.vector.tensor_scalar(
                                    out=tmp[:, :, :ny, :nx],
                                    in0=vgrid[:, :, iy:iy + ny, ix:ix + nx],
                                    scalar1=dwcw_t[:, k * 9 + tap:
                                                   k * 9 + tap + 1],
                                    scalar2=None, op0=OP.mult)
                                nc.vector.tensor_add(
                                    out=dgrid[:, :, oy:oy + ny, ox:ox + nx],
                                    in0=dgrid[:, :, oy:oy + ny, ox:ox + nx],
                                    in1=tmp[:, :, :ny, :nx])
                        nc.vector.tensor_scalar(
                            out=dw[:, k, :, 1:NTOK], in0=dw[:, k, :, 1:NTOK],
                            scalar1=dwcb_t[:, k:k + 1], scalar2=None,
                            op0=OP.add)

                # -- A-iv: per-item agent attention -> y (reuses xy slots) --
                y_nt = [big.tile([128, KC, NTC], f32r, tag="xy", bufs=3,
                                 name=f"y{sb}_{n}") for n in range(NNT)]
                for i in range(NB):
                    ntile, ii = i // INT, i % INT
                    cb = ii * NTOK
                    gi = sb * NB + i
                    qs = q_nt[ntile][:, :, cb:cb + NTOK]
                    kss = k_nt[ntile][:, :, cb:cb + NTOK]

                    qgrid = q_nt[ntile][:, :, cb + 1:cb + NTOK].rearrange(
                        "p k (y x) -> p k y x", y=8)
                    t1 = wk.tile([128, KC, 7, 8], bf16, tag="t1", bufs=2,
                                 name="t1")
                    nc.vector.tensor_add(out=t1[...],
                                         in0=qgrid[:, :, 0:7, :],
                                         in1=qgrid[:, :, 1:8, :])
                    agent = wk.tile([128, KC, AG], bf16, tag="agent", bufs=2,
                                    name="agent")
                    ag4 = agent[:, :, :].rearrange("p k (y x) -> p k y x", y=7)
                    nc.vector.tensor_add(out=ag4[...],
                                         in0=t1[:, :, :, 0:7],
                                         in1=t1[:, :, :, 1:8])

                    # s1 = 4agent @ k^T  [49, 65]; softmax -> A1
                    s1p = ps.tile([AG, NTOK], f32, tag="at", bufs=4,
                                  name="s1p")
                    for k in range(KC):
                        nc.tensor.matmul(s1p[:, :], agent[:, k, :],
                                         kss[:, k, :],
                                         start=(k == 0), stop=(k == KC - 1))
                    s1s = wk.tile([AG, NTOK], f32, tag="s1s", bufs=2,
                                  name="s1s")
                    nc.vector.tensor_add(out=s1s[:, :], in0=s1p[:, :],
                                         in1=pbc_t[:, :])
                    nm1 = wk.tile([AG, 2], f32, tag="nm1", bufs=2, name="nm1")
                    nc.vector.tensor_reduce(nm1[:, 0:1], s1s[:, :], axis=AX.X,
                                            op=OP.max, negate=True)
                    nc.vector.tensor_scalar(out=nm1[:, 1:2], in0=nm1[:, 0:1],
                                            scalar1=SCALE4, scalar2=None,
                                            op0=OP.mult)
                    a1 = wk.tile([AG, NTOK + 1], f32, tag="a1", bufs=2,
                                 name="a1")
                    nc.scalar.activation(a1[:, :NTOK], s1s[:, :], AF.Exp,
                                         bias=nm1[:, 1:2], scale=SCALE4,
                                         accum_out=a1[:, NTOK:NTOK + 1])
                    rr1 = wk.tile([AG, 1], f32, tag="rr1", bufs=2, name="rr1")
                    nc.vector.reciprocal(rr1[:, :], a1[:, NTOK:NTOK + 1])
                    nc.vector.tensor_scalar(out=a1[:, :NTOK], in0=a1[:, :NTOK],
                                            scalar1=rr1[:, :], scalar2=None,
                                            op0=OP.mult)
                    nc.sync.dma_start(out=repap[gi], in_=a1[:, :NTOK])

                    a1tp = ps.tile([NTOK, AG], f32, tag="at", bufs=4,
                                   name="a1tp")
                    nc.tensor.transpose(a1tp[:, :], a1[:, :NTOK],
                                        identf[:AG, :AG])
                    a1t = wk.tile([NTOK, AG], bf16, tag="a1t", bufs=2,
                                  name="a1t")
                    nc.any.tensor_copy(a1t[:, :], a1tp[:, :])

                    # s2 = q @ agent^T  [65, 49]; softmax -> q_attn
                    s2p = ps.tile([NTOK, AG], f32, tag="at", bufs=4,
                                  name="s2p")
                    for k in range(KC):
                        nc.tensor.matmul(s2p[:, :], qs[:, k, :],
                                         agent[:, k, :],
                                         start=(k == 0), stop=(k == KC - 1))
                    s2s = wk.tile([NTOK, AG], f32, tag="s2s", bufs=2,
                                  name="s2s")
                    nc.vector.tensor_add(out=s2s[:, :], in0=s2p[:, :],
                                         in1=abc_t[:, :])
                    nm2 = wk.tile([NTOK, 2], f32, tag="nm2", bufs=2,
                                  name="nm2")
                    nc.vector.tensor_reduce(nm2[:, 0:1], s2s[:, :], axis=AX.X,
                                            op=OP.max, negate=True)
                    nc.vector.tensor_scalar(out=nm2[:, 1:2], in0=nm2[:, 0:1],
                                            scalar1=SCALE4, scalar2=None,
                                            op0=OP.mult)
                    qa = wk.tile([NTOK, AG + 1], f32, tag="qa", bufs=2,
                                 name="qa")
                    nc.scalar.activation(qa[:, :AG], s2s[:, :], AF.Exp,
                                         bias=nm2[:, 1:2], scale=SCALE4,
                                         accum_out=qa[:, AG:AG + 1])
                    rr2 = wk.tile([NTOK, 1], f32, tag="rr2", bufs=2,
                                  name="rr2")
                    nc.vector.reciprocal(rr2[:, :], qa[:, AG:AG + 1])
                    nc.vector.tensor_scalar(out=qa[:, :AG], in0=qa[:, :AG],
                                            scalar1=rr2[:, :], scalar2=None,
                                            op0=OP.mult)
                    qatp = ps.tile([AG, NTOK], f32, tag="at", bufs=4,
                                   name="qatp")
                    nc.tensor.transpose(qatp[:, :], qa[:, :AG],
                                        identf[:NTOK, :NTOK])
                    qat = wk.tile([AG, NTOK], bf16, tag="qat", bufs=2,
                                  name="qat")
                    nc.any.tensor_copy(qat[:, :], qatp[:, :])

                    # v_tm via per-item PE transposes (bf16)
                    vtm = wk.tile([128, DIM], bf16, tag="vtm", bufs=2,
                                  name="vtm")
                    for k in range(KC):
                        vtp = ps.tile([NTOK, 128], bf16, tag="at", bufs=4,
                                      name="vtp")
                        nc.tensor.transpose(vtp[:, :],
                                            v_nt[ntile][:, k, cb:cb + NTOK],
                                            identb[:, :])
                        nc.any.tensor_copy(
                            vtm[:NTOK, k * 128:(k + 1) * 128], vtp[:, :])

                    # agent_v = A1 @ v  [49, 768]
                    av = wk.tile([AG, DIM], bf16, tag="av", bufs=2, name="av")
                    avp1 = ps.tile([AG, 512], f32, tag="at", bufs=4,
                                   name="avp1")
                    nc.tensor.matmul(avp1[:, :], a1t[:, :], vtm[:NTOK, 0:512],
                                     start=True, stop=True)
                    nc.any.tensor_copy(av[:, 0:512], avp1[:, :])
                    avp2 = ps.tile([AG, 256], f32, tag="at", bufs=4,
                                   name="avp2")
                    nc.tensor.matmul(avp2[:, :], a1t[:, :],
                                     vtm[:NTOK, 512:768],
                                     start=True, stop=True)
                    nc.any.tensor_copy(av[:, 512:768], avp2[:, :])

                    # out^T = agent_v^T @ q_attn^T  [768, 65]; y = out + dw
                    op_ = ps.tile([128, KC, NTOK], f32, tag="at", bufs=4,
                                  name="op_")
                    for k in range(KC):
                        nc.tensor.matmul(op_[:, k, :],
                                         av[:, k * 128:(k + 1) * 128],
                                         qat[:, :], start=True, stop=True)
                    nc.vector.tensor_add(out=y_nt[ntile][:, :, cb:cb + NTOK],
                                         in0=op_[:, :, :],
                                         in1=dw_nt[ntile][:, :, ii, :])

                # -- A-v: proj (f32r) + bias -> per-N-tile proj_fm (bf16) --
                pj_nt = [big.tile([128, KC, NTC], bf16, tag="pj", bufs=2,
                                  name=f"pj{sb}_{n}") for n in range(NNT)]
                for ntile in range(NNT):
                    for m in range(KC):
                        mm = ps.tile([128, NTC], f32, tag="mm", bufs=2,
                                     name="mmp")
                        for k in range(KC):
                            nc.tensor.matmul(
                                mm[:, :],
                                wproj_t[:, k, m * 128:(m + 1) * 128],
                                y_nt[ntile][:, k, :],
                                start=(k == 0), stop=(k == KC - 1))
                        nc.vector.tensor_scalar(
                            out=pj_nt[ntile][:, m, :], in0=mm[:, :],
                            scalar1=bproj_t[:, m:m + 1], scalar2=None,
                            op0=OP.add)

                # -- A-vi: xa = x + att (token-major), bounce to DRAM --
                for ntile in range(NNT):
                    for (c0, tp) in _ntiles(NTC, 128):
                        t0 = t0sb + ntile * NTC + c0
                        xa_tm = wk.tile([128, DIM], f32, tag="xa_tm", bufs=3,
                                        name="xa_tm")
                        nc.sync.dma_start(out=xa_tm[:tp, :],
                                          in_=xap[t0: t0 + tp, :])
                        for k in range(KC):
                            tb = ps.tile([128, 128], bf16, tag="tp", bufs=2,
                                         name="tb")
                            nc.tensor.transpose(tb[:tp, :],
                                                pj_nt[ntile][:, k, c0:c0 + tp],
                                                identb[:, :])
                            nc.vector.tensor_add(
                                out=xa_tm[:tp, k * 128:(k + 1) * 128],
                                in0=tb[:tp, :],
                                in1=xa_tm[:tp, k * 128:(k + 1) * 128])
                        nc.sync.dma_start(out=xaap[t0: t0 + tp, :],
                                          in_=xa_tm[:tp, :])

        # ---------------- Phase B ----------------
        pwb = tc.tile_pool(name="pwb", bufs=1)
        pbig2 = tc.tile_pool(name="pbig2", bufs=1)
        pwk2 = tc.tile_pool(name="pwk2", bufs=1)
        pps2 = tc.tile_pool(name="pps2", bufs=1, space="PSUM")
        with pwb as wB, pbig2 as big2, pwk2 as wk2, pps2 as ps2:
            fc1_t = wB.tile([128, KC, MLP], f32r)
            fc2_t = wB.tile([128, MC, DIM], bf16)
            f1_r = fc1_d.ap().rearrange("(k p) m -> k p m", p=128)
            f2_r = fc2_d.ap().rearrange("(k p) m -> k p m", p=128)
            for k in range(KC):
                for pc in range(4):
                    st = wk2.tile([128, DIM], f32, tag="stage", bufs=1,
                                  name="stb")
                    nc.sync.dma_start(out=st[:, :],
                                      in_=f1_r[k][:, pc * DIM:(pc + 1) * DIM])
                    nc.vector.tensor_copy(
                        fc1_t[:, k, pc * DIM:(pc + 1) * DIM], st[:, :])
            for k in range(MC):
                st2 = wk2.tile([128, DIM], f32, tag="stage", bufs=1,
                               name="stb2")
                nc.sync.dma_start(out=st2[:, :], in_=f2_r[k])
                nc.vector.tensor_copy(fc2_t[:, k, :], st2[:, :])

            for (g0, gn) in _ntiles(TOKS, 512):
                xhf2 = big2.tile([128, KC, 512], f32r, tag="xhf2", bufs=1,
                                 name="xhf2")
                for (tt0, tp) in _ntiles(gn, 128):

                    def wr2(k, tps, tt0=tt0, tp=tp):
                        nc.vector.tensor_scalar(
                            out=xhf2[:, k, tt0:tt0 + tp], in0=tps[:, :tp],
                            scalar1=ln2g_t[:, k:k + 1],
                            scalar2=ln2b_t[:, k:k + 1],
                            op0=OP.mult, op1=OP.add)

                    layernorm_to_fm(
                        wk2, ps2, xaap[g0 + tt0: g0 + tt0 + tp, :], tp,
                        ln2g_t, ln2b_t, wr2, identr)

                hts = []
                for m in range(MC):
                    mm = ps2.tile([128, 512], f32, tag="mm", bufs=2,
                                  name="mmf1")
                    for k in range(KC):
                        nc.tensor.matmul(mm[:, :gn],
                                         fc1_t[:, k, m * 128:(m + 1) * 128],
                                         xhf2[:, k, :gn],
                                         start=(k == 0), stop=(k == KC - 1))
                    ht = big2.tile([128, 512], bf16, tag="h", bufs=26,
                                   name=f"h{m}")
                    nc.scalar.activation(ht[:, :gn], mm[:, :gn], AF.Gelu,
                                         bias=fc1b_t[:, m:m + 1])
                    hts.append(ht)

                mlpf = big2.tile([128, KC, 512], bf16, tag="mlpf", bufs=2,
                                 name="mlpf")
                for m in range(KC):
                    mm2 = ps2.tile([128, 512], f32, tag="mm2", bufs=2,
                                   name="mmf2")
                    for k in range(MC):
                        nc.tensor.matmul(mm2[:, :gn],
                                         fc2_t[:, k, m * 128:(m + 1) * 128],
                                         hts[k][:, :gn],
                                         start=(k == 0), stop=(k == MC - 1))
                    nc.vector.tensor_scalar(
                        out=mlpf[:, m, :gn], in0=mm2[:, :gn],
                        scalar1=fc2b_t[:, m:m + 1], scalar2=None, op0=OP.add)

                for (tt0, tp) in _ntiles(gn, 128):
                    out_tm = wk2.tile([128, DIM], f32, tag="out_tm", bufs=2,
                                      name="out_tm")
                    nc.sync.dma_start(out=out_tm[:tp, :],
                                      in_=xaap[g0 + tt0: g0 + tt0 + tp, :])
                    for k in range(KC):
                        tb = ps2.tile([128, 128], bf16, tag="tp", bufs=2,
                                      name="tb2")
                        nc.tensor.transpose(tb[:tp, :],
                                            mlpf[:, k, tt0:tt0 + tp],
                                            identb[:, :])
                        nc.vector.tensor_add(
                            out=out_tm[:tp, k * 128:(k + 1) * 128],
                            in0=tb[:tp, :],
                            in1=out_tm[:tp, k * 128:(k + 1) * 128])
                    nc.sync.dma_start(out=xoap[g0 + tt0: g0 + tt0 + tp, :],
                                      in_=out_tm[:tp, :])

    nc.compile()
    return nc


def _bilinear_7to8(t):
    # (..., 7, 7) -> (..., 8, 8), matches F.interpolate(bilinear, align_corners=False)
    src = np.clip((np.arange(8, dtype=np.float32) + 0.5) * (7.0 / 8.0) - 0.5,
                  0.0, None)
    i0 = np.floor(src).astype(np.int64)
    i1 = np.minimum(i0 + 1, 6)
    f = src - i0

    def lerp(x, axis):
        a = np.take(x, i0, axis=axis)
        b = np.take(x, i1, axis=axis)
        shp = [1] * x.ndim
        shp[axis] = 8
        ff = f.reshape(shp)
        return a * (1.0 - ff) + b * ff

    return lerp(lerp(t, -2), -1)


def _fm_vec(v, chunks):
    # [chunks*128] feature vector -> [128, chunks] feature-major tile layout
    return np.ascontiguousarray(
        np.asarray(v, dtype=np.float32).reshape(chunks, 128).T)


def kernel(**inputs):
    global _PROG
    from concourse.bass_utils import run_bass_kernel_spmd

    if _PROG is None:
        _PROG = _build_program()
    nc = _PROG

    f = {k: np.asarray(v, dtype=np.float32) for k, v in inputs.items()}
    x = f["x"]

    wqkvT = np.ascontiguousarray(f["w_qkv"].T)
    wprojT = np.ascontiguousarray(f["w_proj"].T)
    fc1T = np.ascontiguousarray(f["fc1_w"].T)
    fc2T = np.ascontiguousarray(f["fc2_w"].T)

    # agent->token bias pb [49, 65] and token->agent bias ab [65, 49]
    pb1 = _bilinear_7to8(f["an_bias"]).reshape(1, AG, 64)
    pb2 = (f["ah_bias"] + f["aw_bias"]).reshape(1, AG, 64)
    pb = np.concatenate([f["ac_bias"].reshape(1, AG, 1), pb1 + pb2], axis=-1)
    pbc = np.ascontiguousarray(pb[0] / SCALE4)

    ab1 = _bilinear_7to8(f["na_bias"]).reshape(1, AG, 64).transpose(0, 2, 1)
    ab2 = (f["ha_bias"] + f["wa_bias"]).reshape(1, 64, AG)
    ab = np.concatenate([f["ca_bias"].reshape(1, 1, AG), ab1 + ab2], axis=-2)
    abc = np.ascontiguousarray(ab[0] / SCALE4)

    dwcw = np.ascontiguousarray(
        f["dwc_w"].reshape(DIM, 9).reshape(KC, 128, 9).transpose(1, 0, 2)
    ).reshape(128, KC * 9)

    common = {
        "wqkvT": wqkvT, "wprojT": wprojT, "fc1T": fc1T, "fc2T": fc2T,
        "ln1g": _fm_vec(f["ln1_g"], KC), "ln1b": _fm_vec(f["ln1_b"], KC),
        "ln2g": _fm_vec(f["ln2_g"], KC), "ln2b": _fm_vec(f["ln2_b"], KC),
        "bproj": _fm_vec(f["b_proj"], KC),
        "fc1b": _fm_vec(f["fc1_b"], MC), "fc2b": _fm_vec(f["fc2_b"], KC),
        "dwcw": dwcw, "dwcb": _fm_vec(f["dwc_b"], KC),
        "pbc": pbc.astype(np.float32), "abc": abc.astype(np.float32),
    }
    xs = x.reshape(NCORES, NITEMS * NTOK, DIM)
    in_maps = [dict(common, x=np.ascontiguousarray(xs[c]))
               for c in range(NCORES)]

    res = run_bass_kernel_spmd(nc, in_maps, list(range(NCORES)))

    xout = np.stack([res.results[c]["xout"] for c in range(NCORES)])
    xout = xout.reshape(NCORES * NITEMS, NTOK, DIM)
    rep = np.stack([res.results[c]["rep"] for c in range(NCORES)])
    rep = rep.reshape(NCORES * NITEMS, 1, AG, NTOK)
    return xout.astype(np.float32), rep.astype(np.float32)


# revision 27
# speedup vs baseline: 62.0407x; 62.0407x over previous
"""Trainium2 Bass kernel for the BLIPv2 agent-attention block.

Full (unsharded) inputs in, full outputs out. Data-parallel over the batch
axis across 8 NeuronCores; parameters replicated.

Per-core program (64 batch items, 65 tokens, dim 768):
  Phase A: LN1 -> qkv (f32r matmuls) -> per-item agent attention (bf16
           matmuls) + depthwise 3x3 conv (DVE) -> proj -> xa = x + att
           (bounced to DRAM).
  Phase B: LN2 -> fc1 (f32r) -> exact gelu (ACT) -> fc2 (bf16) -> out =
           xa + mlp.
Layout: token-major for LayerNorm / residuals, feature-major for matmuls,
PE transposes to convert. Buffers are tiled per item-aligned column group
(4 items = 260 columns) so LN / qkv / attention / proj pipeline.
"""

import sys
from contextlib import ExitStack

import numpy as np

sys.path.insert(0, "/opt/trn_rl_repo")

DIM = 768
KC = 6            # DIM / 128
NTOK = 65
AG = 49
MLP = 3072
MC = 24           # MLP / 128
NCORES = 8
NITEMS = 64       # batch items per core
NB = 8            # items per sub-batch
NSB = NITEMS // NB
INT = 4           # items per column group (N-tile)
NTC = INT * NTOK  # N-tile columns (260): >=256 keeps f32r at full rate
NNT = NB // INT   # N-tiles per sub-batch (2)
TSB = NB * NTOK   # tokens per sub-batch (520)
TOKS = NITEMS * NTOK  # tokens per core (4160)
EPS = 1e-5
SCALE4 = (DIM ** -0.5) / 4.0  # folds the 2x2-avg-pool 1/4 into the softmax scale

_PROG = None


def _ntiles(total, step):
    out = []
    o = 0
    while o < total:
        out.append((o, min(step, total - o)))
        o += step
    return out


def _pieces(lo, hi):
    """Split token range [lo, hi) at N-tile boundaries -> (ntile, start, len)."""
    out = []
    while lo < hi:
        nt = lo // NTC
        end = min(hi, (nt + 1) * NTC)
        out.append((nt, lo - nt * NTC, end - lo))
        lo = end
    return out


def _build_program(parts=("A", "B")):
    import concourse.mybir as mybir
    from concourse import bacc
    from concourse.tile import TileContext
    from concourse.masks import make_identity

    f32 = mybir.dt.float32
    f32r = mybir.dt.float32r
    bf16 = mybir.dt.bfloat16
    AF = mybir.ActivationFunctionType
    OP = mybir.AluOpType
    AX = mybir.AxisListType

    nc = bacc.Bacc("TRN2", target_bir_lowering=False, debug=False,
                   num_devices=NCORES)

    x_d = nc.dram_tensor("x", [TOKS, DIM], f32, kind="ExternalInput")
    wqkv_d = nc.dram_tensor("wqkvT", [DIM, 3 * DIM], f32, kind="ExternalInput")
    wproj_d = nc.dram_tensor("wprojT", [DIM, DIM], f32, kind="ExternalInput")
    fc1_d = nc.dram_tensor("fc1T", [DIM, MLP], f32, kind="ExternalInput")
    fc2_d = nc.dram_tensor("fc2T", [MLP, DIM], f32, kind="ExternalInput")
    qkvb_d = nc.dram_tensor("qkvb", [128, 3 * KC], f32, kind="ExternalInput")
    bproj_d = nc.dram_tensor("bproj", [128, KC], f32, kind="ExternalInput")
    fc1b_d = nc.dram_tensor("fc1b", [128, MC], f32, kind="ExternalInput")
    fc2b_d = nc.dram_tensor("fc2b", [128, KC], f32, kind="ExternalInput")
    dwd_d = nc.dram_tensor("dwdiag", [KC * 9 * 128, 128], bf16,
                           kind="ExternalInput")
    dwcb_d = nc.dram_tensor("dwcb", [128, KC], f32, kind="ExternalInput")
    pbc_d = nc.dram_tensor("pbc", [AG, NTOK], f32, kind="ExternalInput")
    abc_d = nc.dram_tensor("abc", [NTOK, AG], f32, kind="ExternalInput")

    xa_d = nc.dram_tensor("xa", [TOKS, DIM], f32)
    xout_d = nc.dram_tensor("xout", [TOKS, DIM], f32, kind="ExternalOutput")
    rep_d = nc.dram_tensor("rep", [NITEMS, AG, NTOK], f32, kind="ExternalOutput")

    xap = x_d.ap()
    xaap = xa_d.ap()
    xoap = xout_d.ap()
    repap = rep_d.ap()

    with TileContext(nc) as tc, ExitStack() as _stk:
        cns = _stk.enter_context(tc.tile_pool(name="consts", bufs=1))
        identf = cns.tile([128, 128], f32)
        make_identity(nc, identf[:, :])
        identr = cns.tile([128, 128], f32r)
        nc.vector.tensor_copy(identr[:, :], identf[:, :])
        identb = cns.tile([128, 128], bf16)
        make_identity(nc, identb[:, :])
        eps_t = cns.tile([128, 1], f32)
        nc.vector.memset(eps_t[:, :], EPS)

        def _load_const(dram, shape):
            t = cns.tile(shape, f32, name=dram.name + "_t")
            nc.sync.dma_start(out=t[...], in_=dram.ap())
            return t

        qkvb_t = _load_const(qkvb_d, [128, 3 * KC])
        bproj_t = _load_const(bproj_d, [128, KC])
        fc1b_t = _load_const(fc1b_d, [128, MC])
        fc2b_t = _load_const(fc2b_d, [128, KC])
        dwdg_t = cns.tile([128, KC * 9, 128], bf16, name="dwdg")
        nc.sync.dma_start(out=dwdg_t[...],
                          in_=dwd_d.ap().rearrange("(t p) m -> p t m", p=128))
        dwcb_t = _load_const(dwcb_d, [128, KC])
        pbc_t = _load_const(pbc_d, [AG, NTOK])
        abc_t = _load_const(abc_d, [NTOK, AG])

        def layernorm_to_fm(wkp, psp, src_slice, tp, write_fn, ident,
                            tptag="tp", tpbufs=2):
            """LN over one token tile (tp tokens), transpose to fm, then
            write_fn(grp, psum_ap) for grp 0 (chunks 0-3) and 1 (chunks 4-5).
            LN affine is pre-folded into the consuming weights."""
            x_tm = wkp.tile([128, DIM], f32, tag="x_tm", bufs=3, name="x_tm")
            nc.sync.dma_start(out=x_tm[:tp, :], in_=src_slice)
            stats = wkp.tile([128, 3, 6], f32, tag="stats", bufs=2,
                             name="stats")
            xg = x_tm[:tp, :].rearrange("p (g d) -> p g d", g=3)
            for g in range(3):
                nc.vector.bn_stats(out=stats[:tp, g, :], in_=xg[:, g, :])
            mv = wkp.tile([128, 2], f32, tag="mv", bufs=2, name="mv")
            nc.vector.bn_aggr(out=mv[:tp, :], in_=stats[:tp, :, :])
            rstd = wkp.tile([128, 2], f32, tag="rstd", bufs=2, name="rstd")
            nc.scalar.activation(rstd[:tp, 1:2], mv[:tp, 1:2], AF.Sqrt,
                                 bias=eps_t[:tp, :])
            nc.vector.reciprocal(rstd[:tp, 1:2], rstd[:tp, 1:2])
            xh_tm = wkp.tile([128, DIM], f32r, tag="xh_tm", bufs=2,
                             name="xh_tm")
            nc.vector.tensor_scalar(out=xh_tm[:tp, :], in0=x_tm[:tp, :],
                                    scalar1=mv[:tp, 0:1],
                                    scalar2=rstd[:tp, 1:2],
                                    op0=OP.subtract, op1=OP.mult)
            for grp, nk in ((0, 4), (1, 2)):
                tps = psp.tile([128, 4, 128], f32r, tag=tptag, bufs=tpbufs,
                               name="tps")
                for j in range(nk):
                    k = grp * 4 + j
                    nc.tensor.transpose(tps[:, j, :tp],
                                        xh_tm[:tp, k * 128:(k + 1) * 128],
                                        ident[:tp, :tp])
                write_fn(grp, nk, tps)

        # ---------------- Phase A ----------------
        pwa = tc.tile_pool(name="pwa", bufs=1)
        pbig = tc.tile_pool(name="pbig", bufs=1)
        pwk = tc.tile_pool(name="pwk", bufs=1)
        pps = tc.tile_pool(name="pps", bufs=1, space="PSUM")
        with pwa as wA, pbig as big, pwk as wk, pps as ps:
            wqkv_ts = [wA.tile([128, 3 * DIM], f32r, name=f"wqkv{k}")
                       for k in range(KC)]
            wproj_ts = [wA.tile([128, DIM], f32r, name=f"wproj{k}")
                        for k in range(KC)]
            wq_r = wqkv_d.ap().rearrange("(k p) m -> k p m", p=128)
            wp_r = wproj_d.ap().rearrange("(k p) m -> k p m", p=128)
            for k in (range(KC) if "A" in parts else ()):
                for pc in range(3):
                    st = wk.tile([128, DIM], f32, tag="stage", bufs=2,
                                 name="st")
                    nc.sync.dma_start(out=st[:, :],
                                      in_=wq_r[k][:, pc * DIM:(pc + 1) * DIM])
                    nc.vector.tensor_copy(
                        wqkv_ts[k][:, pc * DIM:(pc + 1) * DIM], st[:, :])
                st2 = wk.tile([128, DIM], f32, tag="stage", bufs=2, name="st2")
                nc.sync.dma_start(out=st2[:, :], in_=wp_r[k])
                nc.vector.tensor_copy(wproj_ts[k][:, :], st2[:, :])

            for sb in (range(NSB) if "A" in parts else ()):
                t0sb = sb * TSB

                # -- A-i: LN1 + transpose into per-N-tile fm buffers --
                xh_nt = [big.tile([128, KC, NTC], f32r, tag="xy", bufs=3,
                                  name=f"xh{sb}_{n}") for n in range(NNT)]
                for ntile in range(NNT):
                    for (c0, tp) in _ntiles(NTC, 128):
                        def wr(grp, nk, tps, ntile=ntile, c0=c0, tp=tp):
                            nc.vector.tensor_copy(
                                xh_nt[ntile][:, grp * 4:grp * 4 + nk,
                                             c0:c0 + tp],
                                tps[:, :nk, :tp])

                        t0 = t0sb + ntile * NTC + c0
                        layernorm_to_fm(wk, ps, xap[t0: t0 + tp, :], tp,
                                        wr, identr, tptag="at", tpbufs=6)

                # -- A-ii: qkv matmuls (f32r) -> per-N-tile q/k/v bf16 --
                q_nt = [big.tile([128, KC, NTC], bf16, tag="q", bufs=2,
                                 name=f"q{sb}_{n}") for n in range(NNT)]
                k_nt = [big.tile([128, KC, NTC], bf16, tag="k", bufs=2,
                                 name=f"k{sb}_{n}") for n in range(NNT)]
                v_nt = [big.tile([128, KC, NTC], bf16, tag="v", bufs=2,
                                 name=f"v{sb}_{n}") for n in range(NNT)]
                for ntile in range(NNT):
                    for m in range(3 * KC):
                        mm = ps.tile([128, NTC], f32, tag="mm", bufs=2,
                                     name="mmq")
                        for k in range(KC):
                            nc.tensor.matmul(
                                mm[:, :],
                                wqkv_ts[k][:, m * 128:(m + 1) * 128],
                                xh_nt[ntile][:, k, :],
                                start=(k == 0), stop=(k == KC - 1))
                        dst = (q_nt, k_nt, v_nt)[m // KC][ntile]
                        nc.any.tensor_scalar(
                            out=dst[:, m % KC, :], in0=mm[:, :],
                            scalar1=qkvb_t[:, m:m + 1], scalar2=None,
                            op0=OP.add)

                # -- A-iii: depthwise 3x3 conv on PE (diagonal matmuls) --
                dw_nt = []
                for ntile in range(NNT):
                    dw = big.tile([128, KC, INT, 64], bf16, tag="dw",
                                  bufs=2, name=f"dw{sb}_{ntile}")
                    dw_nt.append(dw)
                    for k in range(KC):
                        dwps = ps.tile([128, INT, NTOK], f32, tag="mm",
                                       bufs=2, name="dwps")
                        vgrid = v_nt[ntile][:, k, :].rearrange(
                            "p (i n) -> p i n", i=INT)[:, :, 1:NTOK].rearrange(
                            "p i (y x) -> p i y x", y=8)
                        ogrid = dwps[:, :, 1:NTOK].rearrange(
                            "p i (y x) -> p i y x", y=8)
                        for dy in (-1, 0, 1):
                            for dx in (-1, 0, 1):
                                ny, nx = 8 - abs(dy), 8 - abs(dx)
                                oy, ox = max(0, -dy), max(0, -dx)
                                iy, ix = max(0, dy), max(0, dx)
                                tap = (dy + 1) * 3 + (dx + 1)
                                nc.tensor.matmul(
                                    ogrid[:, :, oy:oy + ny, ox:ox + nx],
                                    dwdg_t[:, k * 9 + tap, :],
                                    vgrid[:, :, iy:iy + ny, ix:ix + nx],
                                    start=(tap == 0), stop=(tap == 8))
                        nc.any.tensor_scalar(
                            out=dw[:, k, :, :],
                            in0=dwps[:, :, 1:NTOK],
                            scalar1=dwcb_t[:, k:k + 1], scalar2=None,
                            op0=OP.add)

                # -- A-iv: agent attention, stage-major across the 8 items
                # so the in-order engines interleave independent item chains --
                y_nt = [big.tile([128, KC, NTC], f32r, tag="xy", bufs=3,
                                 name=f"y{sb}_{n}") for n in range(NNT)]

                agents, a1ts, qats, vtms, avs = {}, {}, {}, {}, {}

                # S0: 2x2 sum-pool of q spatial grid -> 4*agent (bf16)
                for i in range(NB):
                    ntile, cb = i // INT, (i % INT) * NTOK
                    qgrid = q_nt[ntile][:, :, cb + 1:cb + NTOK].rearrange(
                        "p k (y x) -> p k y x", y=8)
                    t1 = wk.tile([128, KC, 7, 8], bf16, tag="t1", bufs=2,
                                 name="t1")
                    nc.vector.tensor_add(out=t1[...],
                                         in0=qgrid[:, :, 0:7, :],
                                         in1=qgrid[:, :, 1:8, :])
                    agent = wk.tile([128, KC, AG], bf16, tag="agent", bufs=9,
                                    name="agent")
                    ag4 = agent[:, :, :].rearrange("p k (y x) -> p k y x", y=7)
                    nc.vector.tensor_add(out=ag4[...],
                                         in0=t1[:, :, :, 0:7],
                                         in1=t1[:, :, :, 1:8])
                    agents[i] = agent

                # S1: s1 = 4agent @ k^T -> softmax -> A1 (rep out) -> A1^T
                for i in range(NB):
                    ntile, cb = i // INT, (i % INT) * NTOK
                    gi = sb * NB + i
                    kss = k_nt[ntile][:, :, cb:cb + NTOK]
                    s1p = ps.tile([AG, NTOK], f32, tag="at", bufs=6,
                                  name="s1p")
                    for k in range(KC):
                        nc.tensor.matmul(s1p[:, :], agents[i][:, k, :],
                                         kss[:, k, :],
                                         start=(k == 0), stop=(k == KC - 1))
                    s1s = wk.tile([AG, NTOK], f32, tag="s1s", bufs=2,
                                  name="s1s")
                    nc.vector.tensor_add(out=s1s[:, :], in0=s1p[:, :],
                                         in1=pbc_t[:, :])
                    a1 = wk.tile([AG, NTOK + 1], f32, tag="a1", bufs=2,
                                 name="a1")
                    nc.scalar.activation(a1[:, :NTOK], s1s[:, :], AF.Exp,
                                         scale=SCALE4,
                                         accum_out=a1[:, NTOK:NTOK + 1])
                    a1n = wk.tile([AG, NTOK], f32, tag="a1n", bufs=9,
                                  name="a1n")
                    nc.gpsimd.normalize_recip(a1n[:, :], a1[:, :NTOK],
                                              a1[:, NTOK:NTOK + 1])
                    nc.sync.dma_start(out=repap[gi], in_=a1n[:, :])
                    a1ts[i] = a1n

                # S1b: A1 transposes (separate loop keeps the PE stream free)
                for i in range(NB):
                    a1tp = ps.tile([NTOK, AG], f32, tag="at", bufs=6,
                                   name="a1tp")
                    nc.tensor.transpose(a1tp[:, :], a1ts[i][:, :],
                                        identf[:AG, :AG])
                    a1t = wk.tile([NTOK, AG], bf16, tag="a1t", bufs=9,
                                  name="a1t")
                    nc.any.tensor_copy(a1t[:, :], a1tp[:, :])
                    a1ts[i] = a1t

                # S2: s2 = q @ agent^T -> softmax -> q_attn^T; v_tm transposes
                for i in range(NB):
                    ntile, cb = i // INT, (i % INT) * NTOK
                    qs = q_nt[ntile][:, :, cb:cb + NTOK]
                    s2p = ps.tile([NTOK, AG], f32, tag="at", bufs=6,
                                  name="s2p")
                    for k in range(KC):
                        nc.tensor.matmul(s2p[:, :], qs[:, k, :],
                                         agents[i][:, k, :],
                                         start=(k == 0), stop=(k == KC - 1))
                    s2s = wk.tile([NTOK, AG], f32, tag="s2s", bufs=2,
                                  name="s2s")
                    nc.vector.tensor_add(out=s2s[:, :], in0=s2p[:, :],
                                         in1=abc_t[:, :])
                    qa = wk.tile([NTOK, AG + 1], f32, tag="qa", bufs=2,
                                 name="qa")
                    nc.scalar.activation(qa[:, :AG], s2s[:, :], AF.Exp,
                                         scale=SCALE4,
                                         accum_out=qa[:, AG:AG + 1])
                    qan = wk.tile([NTOK, AG], f32, tag="qan", bufs=9,
                                  name="qan")
                    nc.gpsimd.normalize_recip(qan[:, :], qa[:, :AG],
                                              qa[:, AG:AG + 1])
                    qats[i] = qan

                # S2b: q_attn transposes + v_tm builds
                for i in range(NB):
                    ntile, cb = i // INT, (i % INT) * NTOK
                    qatp = ps.tile([AG, NTOK], f32, tag="at", bufs=6,
                                   name="qatp")
                    nc.tensor.transpose(qatp[:, :], qats[i][:, :],
                                        identf[:NTOK, :NTOK])
                    qat = wk.tile([AG, NTOK], bf16, tag="qat", bufs=9,
                                  name="qat")
                    nc.any.tensor_copy(qat[:, :], qatp[:, :])
                    qats[i] = qat

                    vtm = wk.tile([128, DIM], bf16, tag="vtm", bufs=8,
                                  name="vtm")
                    for half in range(2):
                        vtp = ps.tile([NTOK, 3, 128], bf16, tag="at", bufs=6,
                                      name="vtp")
                        for j in range(3):
                            k = half * 3 + j
                            nc.tensor.transpose(
                                vtp[:, j, :],
                                v_nt[ntile][:, k, cb:cb + NTOK],
                                identb[:, :])
                        nc.any.tensor_copy(
                            vtm[:NTOK, half * 384:(half + 1) * 384],
                            vtp[:, :, :])
                    vtms[i] = vtm

                # S3: agent_v = A1 @ v
                for i in range(NB):
                    av = wk.tile([AG, DIM], bf16, tag="av", bufs=8, name="av")
                    avp1 = ps.tile([AG, 512], f32, tag="at", bufs=6,
                                   name="avp1")
                    nc.tensor.matmul(avp1[:, :], a1ts[i][:, :],
                                     vtms[i][:NTOK, 0:512],
                                     start=True, stop=True)
                    nc.any.tensor_copy(av[:, 0:512], avp1[:, :])
                    avp2 = ps.tile([AG, 256], f32, tag="at", bufs=6,
                                   name="avp2")
                    nc.tensor.matmul(avp2[:, :], a1ts[i][:, :],
                                     vtms[i][:NTOK, 512:768],
                                     start=True, stop=True)
                    nc.any.tensor_copy(av[:, 512:768], avp2[:, :])
                    avs[i] = av

                # S4: out^T = agent_v^T @ q_attn^T; y = out + dw
                for i in range(NB):
                    ntile, ii = i // INT, i % INT
                    cb = ii * NTOK
                    op_ = ps.tile([128, KC, NTOK], f32, tag="at", bufs=6,
                                  name="op_")
                    for k in range(KC):
                        nc.tensor.matmul(op_[:, k, :],
                                         avs[i][:, k * 128:(k + 1) * 128],
                                         qats[i][:, :], start=True, stop=True)
                    nc.vector.tensor_add(
                        out=y_nt[ntile][:, :, cb + 1:cb + NTOK],
                        in0=op_[:, :, 1:NTOK],
                        in1=dw_nt[ntile][:, :, ii, :])
                    nc.any.tensor_copy(y_nt[ntile][:, :, cb:cb + 1],
                                       op_[:, :, 0:1])

                # -- A-v: proj (f32r) + bias -> per-N-tile proj_fm (bf16) --
                pj_nt = [big.tile([128, KC, NTC], bf16, tag="pj", bufs=2,
                                  name=f"pj{sb}_{n}") for n in range(NNT)]
                for ntile in range(NNT):
                    for m in range(KC):
                        mm = ps.tile([128, NTC], f32, tag="mm", bufs=2,
                                     name="mmp")
                        for k in range(KC):
                            nc.tensor.matmul(
                                mm[:, :],
                                wproj_ts[k][:, m * 128:(m + 1) * 128],
                                y_nt[ntile][:, k, :],
                                start=(k == 0), stop=(k == KC - 1))
                        nc.any.tensor_scalar(
                            out=pj_nt[ntile][:, m, :], in0=mm[:, :],
                            scalar1=bproj_t[:, m:m + 1], scalar2=None,
                            op0=OP.add)

                # -- A-vi: xa = x + att (token-major), bounce to DRAM --
                for ntile in range(NNT):
                    for (c0, tp) in _ntiles(NTC, 128):
                        t0 = t0sb + ntile * NTC + c0
                        xa_tm = wk.tile([128, DIM], f32, tag="xa_tm", bufs=3,
                                        name="xa_tm")
                        nc.sync.dma_start(out=xa_tm[:tp, :],
                                          in_=xap[t0: t0 + tp, :])
                        for grp, nk in ((0, 4), (1, 2)):
                            tb = ps.tile([128, 4, 128], bf16, tag="at",
                                         bufs=6, name="tb")
                            for j in range(nk):
                                k = grp * 4 + j
                                nc.tensor.transpose(
                                    tb[:tp, j, :],
                                    pj_nt[ntile][:, k, c0:c0 + tp],
                                    identb[:, :])
                            d0 = grp * 512
                            nc.vector.tensor_add(
                                out=xa_tm[:tp, d0:d0 + nk * 128],
                                in0=tb[:tp, :nk, :].rearrange(
                                    "p k n -> p (k n)"),
                                in1=xa_tm[:tp, d0:d0 + nk * 128])
                        nc.sync.dma_start(out=xaap[t0: t0 + tp, :],
                                          in_=xa_tm[:tp, :])

        # ---------------- Phase B ----------------
        pwb = tc.tile_pool(name="pwb", bufs=1)
        pbig2 = tc.tile_pool(name="pbig2", bufs=1)
        pwk2 = tc.tile_pool(name="pwk2", bufs=1)
        pps2 = tc.tile_pool(name="pps2", bufs=1, space="PSUM")
        with pwb as wB, pbig2 as big2, pwk2 as wk2, pps2 as ps2:
            fc1_ts = [wB.tile([128, MLP], f32r, name=f"fc1_{k}")
                      for k in range(KC)]
            fc2_ts = [wB.tile([128, DIM], bf16, name=f"fc2_{k}")
                      for k in range(MC)]
            f1_r = fc1_d.ap().rearrange("(k p) m -> k p m", p=128)
            f2_r = fc2_d.ap().rearrange("(k p) m -> k p m", p=128)
            for k in (range(KC) if "B" in parts else ()):
                for pc in range(4):
                    st = wk2.tile([128, DIM], f32, tag="stage", bufs=2,
                                  name="stb")
                    nc.sync.dma_start(out=st[:, :],
                                      in_=f1_r[k][:, pc * DIM:(pc + 1) * DIM])
                    nc.vector.tensor_copy(
                        fc1_ts[k][:, pc * DIM:(pc + 1) * DIM], st[:, :])
            for k in (range(MC) if "B" in parts else ()):
                st2 = wk2.tile([128, DIM], f32, tag="stage", bufs=2,
                               name="stb2")
                nc.sync.dma_start(out=st2[:, :], in_=f2_r[k])
                nc.vector.tensor_copy(fc2_ts[k][:, :], st2[:, :])

            for (g0, gn) in (_ntiles(TOKS, 512) if "B" in parts else ()):
                xhf2 = big2.tile([128, KC, 512], f32r, tag="xhf2", bufs=2,
                                 name="xhf2")
                for (tt0, tp) in _ntiles(gn, 128):

                    def wr2(grp, nk, tps, tt0=tt0, tp=tp):
                        nc.vector.tensor_copy(
                            xhf2[:, grp * 4:grp * 4 + nk, tt0:tt0 + tp],
                            tps[:, :nk, :tp])

                    layernorm_to_fm(
                        wk2, ps2, xaap[g0 + tt0: g0 + tt0 + tp, :], tp,
                        wr2, identr)

                hts = []
                for m in range(MC):
                    mm = ps2.tile([128, 512], f32, tag="mm", bufs=3,
                                  name="mmf1")
                    for k in range(KC):
                        nc.tensor.matmul(mm[:, :gn],
                                         fc1_ts[k][:, m * 128:(m + 1) * 128],
                                         xhf2[:, k, :gn],
                                         start=(k == 0), stop=(k == KC - 1))
                    ht = big2.tile([128, 512], bf16, tag="h", bufs=26,
                                   name=f"h{m}")
                    nc.scalar.activation(ht[:, :gn], mm[:, :gn], AF.Gelu,
                                         bias=fc1b_t[:, m:m + 1])
                    hts.append(ht)

                mlpf = big2.tile([128, KC, 512], bf16, tag="mlpf", bufs=1,
                                 name="mlpf")
                for m in range(KC):
                    mm2 = ps2.tile([128, 512], f32, tag="mm2", bufs=3,
                                   name="mmf2")
                    for k in range(MC):
                        nc.tensor.matmul(mm2[:, :gn],
                                         fc2_ts[k][:, m * 128:(m + 1) * 128],
                                         hts[k][:, :gn],
                                         start=(k == 0), stop=(k == MC - 1))
                    nc.vector.tensor_scalar(
                        out=mlpf[:, m, :gn], in0=mm2[:, :gn],
                        scalar1=fc2b_t[:, m:m + 1], scalar2=None, op0=OP.add)

                for (tt0, tp) in _ntiles(gn, 128):
                    out_tm = wk2.tile([128, DIM], f32, tag="out_tm", bufs=2,
                                      name="out_tm")
                    nc.sync.dma_start(out=out_tm[:tp, :],
                                      in_=xaap[g0 + tt0: g0 + tt0 + tp, :])
                    for grp, nk in ((0, 4), (1, 2)):
                        tb = ps2.tile([128, 4, 128], bf16, tag="tp", bufs=2,
                                      name="tb2")
                        for j in range(nk):
                            k = grp * 4 + j
                            nc.tensor.transpose(tb[:tp, j, :],
                                                mlpf[:, k, tt0:tt0 + tp],
                                                identb[:, :])
                        d0 = grp * 512
                        nc.vector.tensor_add(
                            out=out_tm[:tp, d0:d0 + nk * 128],
                            in0=tb[:tp, :nk, :].rearrange("p k n -> p (k n)"),
                            in1=out_tm[:tp, d0:d0 + nk * 128])
                    nc.sync.dma_start(out=xoap[g0 + tt0: g0 + tt0 + tp, :],
                                      in_=out_tm[:tp, :])

    nc.compile()
    return nc


def _bilinear_7to8(t):
    # (..., 7, 7) -> (..., 8, 8), matches F.interpolate(bilinear, align_corners=False)
    src = np.clip((np.arange(8, dtype=np.float32) + 0.5) * (7.0 / 8.0) - 0.5,
                  0.0, None)
    i0 = np.floor(src).astype(np.int64)
    i1 = np.minimum(i0 + 1, 6)
    f = src - i0

    def lerp(x, axis):
        a = np.take(x, i0, axis=axis)
        b = np.take(x, i1, axis=axis)
        shp = [1] * x.ndim
        shp[axis] = 8
        ff = f.reshape(shp)
        return a * (1.0 - ff) + b * ff

    return lerp(lerp(t, -2), -1)


def _fm_vec(v, chunks):
    # [chunks*128] feature vector -> [128, chunks] feature-major tile layout
    return np.ascontiguousarray(
        np.asarray(v, dtype=np.float32).reshape(chunks, 128).T)


def kernel(**inputs):
    global _PROG
    from concourse.bass_utils import run_bass_kernel_spmd

    if _PROG is None:
        _PROG = _build_program()
    nc = _PROG

    f = {k: np.asarray(v, dtype=np.float32) for k, v in inputs.items()}
    x = f["x"]

    # fold LN affine transforms into the consuming weights / biases
    wqkvT = np.ascontiguousarray(f["w_qkv"].T * f["ln1_g"][:, None])
    qkvb = f["w_qkv"] @ f["ln1_b"]                     # [2304]
    wprojT = np.ascontiguousarray(f["w_proj"].T)
    fc1T = np.ascontiguousarray(f["fc1_w"].T * f["ln2_g"][:, None])
    fc1b = f["fc1_b"] + f["fc1_w"] @ f["ln2_b"]        # [3072]
    fc2T = np.ascontiguousarray(f["fc2_w"].T)

    # agent->token bias pb [49, 65] and token->agent bias ab [65, 49]
    pb1 = _bilinear_7to8(f["an_bias"]).reshape(1, AG, 64)
    pb2 = (f["ah_bias"] + f["aw_bias"]).reshape(1, AG, 64)
    pb = np.concatenate([f["ac_bias"].reshape(1, AG, 1), pb1 + pb2], axis=-1)
    pbc = np.ascontiguousarray(pb[0] / SCALE4)

    ab1 = _bilinear_7to8(f["na_bias"]).reshape(1, AG, 64).transpose(0, 2, 1)
    ab2 = (f["ha_bias"] + f["wa_bias"]).reshape(1, 64, AG)
    ab = np.concatenate([f["ca_bias"].reshape(1, 1, AG), ab1 + ab2], axis=-2)
    abc = np.ascontiguousarray(ab[0] / SCALE4)

    import ml_dtypes
    w9 = f["dwc_w"].reshape(DIM, 9)                    # [768, 9]
    dwdiag = np.zeros((KC, 9, 128, 128), np.float32)
    for k in range(KC):
        for t in range(9):
            np.fill_diagonal(dwdiag[k, t], w9[k * 128:(k + 1) * 128, t])
    dwdiag = dwdiag.reshape(KC * 9 * 128, 128).astype(ml_dtypes.bfloat16)

    common = {
        "wqkvT": wqkvT, "wprojT": wprojT, "fc1T": fc1T, "fc2T": fc2T,
        "qkvb": _fm_vec(qkvb, 3 * KC),
        "bproj": _fm_vec(f["b_proj"], KC),
        "fc1b": _fm_vec(fc1b, MC), "fc2b": _fm_vec(f["fc2_b"], KC),
        "dwdiag": dwdiag, "dwcb": _fm_vec(f["dwc_b"], KC),
        "pbc": pbc.astype(np.float32), "abc": abc.astype(np.float32),
    }
    xs = x.reshape(NCORES, NITEMS * NTOK, DIM)
    in_maps = [dict(common, x=np.ascontiguousarray(xs[c]))
               for c in range(NCORES)]

    res = run_bass_kernel_spmd(nc, in_maps, list(range(NCORES)))

    xout = np.stack([res.results[c]["xout"] for c in range(NCORES)])
    xout = xout.reshape(NCORES * NITEMS, NTOK, DIM)
    rep = np.stack([res.results[c]["rep"] for c in range(NCORES)])
    rep = rep.reshape(NCORES * NITEMS, 1, AG, NTOK)
    return xout.astype(np.float32), rep.astype(np.float32)
